# revision 2
# baseline (speedup 1.0000x reference)
"""BiSeparableConv (ternary depthwise 3x3 + ternary pointwise 1x1) on 8 TRN2 cores.

Math (folded on host):
  m_dw[c]  = max(mean|w_dw[c]|, EPS)            per-channel depthwise scale
  u_dw     = clip(round(w_dw / m_dw), -1, 1)    ternary taps
  M_pw     = max(mean|w_pw|, EPS)               global pointwise scale
  u_pw     = clip(round(w_pw / M_pw), -1, 1)
  y[n,o,s] = sum_c Wt[o,c] * z[n,c,s]           Wt = M_pw * u_pw * m_dw[c]
  z[n,c,s] = sum_t u_dw[c,t] * x[n,c,s+d_t]     9-tap depthwise, pad=1

Device (per core, 2 images, fp16 compute, fp32 PSUM accum):
  - x host-padded to 58x58 rows (zero border), fp16.
  - depthwise split per image between DVE and PE:
      DVE : rows [0, D_IMG[img]) via 1 tensor_scalar mul + 8 fused
            scalar_tensor_tensor FMAs (one pass per tap)
      PE  : rows [D_IMG[img], 56) via diagonal-matrix matmuls
            (9 taps accumulated in PSUM), ACT copies PSUM->SBUF
  - pointwise: 3x3 blocked matmul (K=384) over chunks of 8 rows,
    PSUM chunk-pairs copied out by ACT; per-image chunks that depend
    only on PE-computed z are issued first (DVE gets maximum slack).
  - y written fp16, host upcasts to fp32.
"""

import numpy as np

# ---------------------------------------------------------------- constants
N_CORES = 8
IMGS = 16
IMG_PER_CORE = 2
C = 384
BLK = 3          # channel blocks of 128
H = W = 56
WP = 58          # padded row width / padded row count
PADLEN = WP * WP           # 3364
SLEN = H * W               # 3136
EPS = 1e-5

D_IMG = (38, 20)   # depthwise rows [0, d) per image handled by DVE
XS_IMG = (36, 18)  # x DMA band-split row per image
CHUNK = 8          # pointwise / PE-dw chunk rows

TAPS = [(dh, dw) for dh in range(3) for dw in range(3)]
DELTA = {t: WP * t[0] + t[1] for t in TAPS}

_cache = {}


def _build(nc_mod, reps=1):
    bass, bacc, tile, mybir = nc_mod
    f16 = mybir.dt.float16
    f32 = mybir.dt.float32
    ALU = mybir.AluOpType

    nc = bacc.Bacc(
        "TRN2", target_bir_lowering=False, debug=False, num_devices=N_CORES
    )

    x_d = nc.dram_tensor("x", [IMG_PER_CORE * BLK, 128, PADLEN], f16,
                         kind="ExternalInput")
    wt_d = nc.dram_tensor("wt", [128, BLK * BLK * 128], f16,
                          kind="ExternalInput")
    dg_d = nc.dram_tensor("dg", [128, BLK * 9 * 128], f16,
                          kind="ExternalInput")
    sc_d = nc.dram_tensor("sc", [128, BLK * 9], f32, kind="ExternalInput")
    y_d = nc.dram_tensor("y", [IMG_PER_CORE * BLK, 128, SLEN], f16,
                         kind="ExternalOutput")

    with tile.TileContext(nc) as tc:
        with (
            tc.tile_pool(name="xa", bufs=1) as xa_pool,
            tc.tile_pool(name="zz", bufs=1) as z_pool,
            tc.tile_pool(name="yy", bufs=1) as y_pool,
            tc.tile_pool(name="wts", bufs=1) as w_pool,
            tc.tile_pool(name="dwps", bufs=2, space="PSUM") as dwps,
            tc.tile_pool(name="pwps", bufs=3, space="PSUM") as pwps,
        ):
            xa = [xa_pool.tile([128, PADLEN], f16, tag=f"xa{u}", name=f"xa{u}")
                  for u in range(6)]
            z = [z_pool.tile([128, PADLEN], f16, tag=f"z{u}", name=f"z{u}")
                 for u in range(6)]
            ym = [y_pool.tile([128, BLK * SLEN], f16, tag=f"ym{i}",
                              name=f"ym{i}") for i in range(2)]
            wt = w_pool.tile([128, BLK * BLK * 128], f16, tag="wt", name="wt")
            dg = w_pool.tile([128, BLK * 9 * 128], f16, tag="dg", name="dg")
            sc = w_pool.tile([128, BLK * 9], f32, tag="sc", name="sc")

            def scal(b, t):
                return sc[:, b * 9 + TAPS.index(t), None]

            def wt_ap(kb, mb):
                i = kb * BLK + mb
                return wt[:, 128 * i:128 * (i + 1)]

            def dg_ap(b, t):
                i = b * 9 + TAPS.index(t)
                return dg[:, 128 * i:128 * (i + 1)]

            def unit(img, b):
                return img * BLK + b

            for _rep in range(reps):
                # ---- DMA in: ordered for earliest PE/DVE starts
                DG = 9 * 128

                def xs_bytes(img):
                    return WP * XS_IMG[img]

                def dma_x(u, band):
                    img = u // BLK
                    xs = xs_bytes(img)
                    if band == 1:
                        nc.sync.dma_start(out=xa[u][:, :xs], in_=x_d[u][:, :xs])
                    else:
                        nc.sync.dma_start(out=xa[u][:, xs:], in_=x_d[u][:, xs:])

                nc.sync.dma_start(out=dg[:, :DG], in_=dg_d[:, :DG])
                dma_x(0, 2)
                nc.sync.dma_start(out=dg[:, DG:2 * DG], in_=dg_d[:, DG:2 * DG])
                dma_x(0, 1)
                dma_x(1, 2)
                nc.sync.dma_start(out=dg[:, 2 * DG:], in_=dg_d[:, 2 * DG:])
                nc.sync.dma_start(out=sc[:], in_=sc_d[:])
                dma_x(1, 1)
                dma_x(2, 2)
                dma_x(2, 1)
                dma_x(3, 2)
                nc.sync.dma_start(out=wt[:], in_=wt_d[:])
                dma_x(4, 2)
                dma_x(5, 2)
                dma_x(3, 1)
                dma_x(4, 1)
                dma_x(5, 1)

                # PE warmup: burn the pstate ramp while x bands land
                wps = dwps.tile([128, 512], f32, tag="dwps", name="dwps")
                for wi in range(3):
                    nc.tensor.matmul(wps[:, :384], dg[:, :128],
                                     dg[:, :3 * 128],
                                     start=(wi == 0), stop=(wi == 2))

                # ---- DVE: rows [0, d) per (img, blk) via fused FMA chain
                def stt_chain(img, b):
                    d = D_IMG[img]
                    u = unit(img, b)
                    p1 = WP * d
                    zr = z[u][:, 0:p1]
                    t0 = TAPS[0]
                    dlt = DELTA[t0]
                    nc.vector.tensor_scalar_mul(
                        zr, xa[u][:, dlt:p1 + dlt], scal(b, t0))
                    for t in TAPS[1:]:
                        dlt = DELTA[t]
                        nc.vector.scalar_tensor_tensor(
                            zr, xa[u][:, dlt:p1 + dlt], scal(b, t), zr,
                            ALU.mult, ALU.add)

                for img in range(2):
                    for b in range(BLK):
                        stt_chain(img, b)

                # ---- PE: dw tail rows [d, 56) via diagonal matmuls
                def pe_dw(img, b):
                    d = D_IMG[img]
                    u = unit(img, b)
                    x3 = xa[u].rearrange("p (h w) -> p h w", w=WP)
                    z3 = z[u].rearrange("p (h w) -> p h w", w=WP)
                    for r0 in range(CHUNK * (d // CHUNK), H, CHUNK):
                        lo = max(r0, d)
                        nrow = min(CHUNK, H - lo)
                        ps = dwps.tile([128, 512], f32, tag="dwps",
                                       name="dwps")
                        dst = ps[:, :nrow * W]
                        for i, t in enumerate(TAPS):
                            dh, dw = t
                            rhs = x3[:, lo + dh:lo + dh + nrow, dw:dw + W]
                            nc.tensor.matmul(dst, dg_ap(b, t), rhs,
                                             start=(i == 0), stop=(i == 8))
                        nc.scalar.copy(z3[:, lo:lo + nrow, 0:W], dst)

                for img in range(2):
                    for b in range(BLK):
                        pe_dw(img, b)

                # ---- pointwise: chunk pairs into 2-bank PSUM, one ACT copy
                def pw_group(img, chunks):
                    for mb in range(BLK):
                        ps = pwps.tile([128, 1024], f32, tag="pwps",
                                       name="pwps")
                        for half, r0 in enumerate(chunks):
                            nrow = min(CHUNK, H - r0)
                            dst = ps[:, 512 * half:512 * half + nrow * W]
                            for kb in range(BLK):
                                zk = z[unit(img, kb)].rearrange(
                                    "p (h w) -> p h w", w=WP)
                                rhs = zk[:, r0:r0 + nrow, 0:W]
                                nc.tensor.matmul(dst, wt_ap(kb, mb), rhs,
                                                 start=(kb == 0),
                                                 stop=(kb == 2))
                        r0 = chunks[0]
                        yo = mb * SLEN
                        if len(chunks) == 2:
                            src_ap = ps.rearrange("p (a q) -> p a q", q=512)[
                                :, 0:2, 0:CHUNK * W]
                            dst_ap = ym[img][
                                :, yo + W * r0:yo + W * r0 + 2 * CHUNK * W
                            ].rearrange("p (a q) -> p a q", q=CHUNK * W)
                            nc.scalar.copy(dst_ap, src_ap)
                        else:
                            dst1 = ym[img][:, yo + W * r0:yo + W * (r0 + CHUNK)]
                            nc.scalar.copy(dst1, ps[:, :CHUNK * W])

                def pw_order(img):
                    d = D_IMG[img]
                    allc = list(range(0, H, CHUNK))
                    tail = [r0 for r0 in allc if r0 >= d]
                    rest = [r0 for r0 in allc if r0 < d]
                    groups = []
                    for lst in (tail, rest):
                        for i in range(0, len(lst) - 1, 2):
                            groups.append([lst[i], lst[i + 1]])
                        if len(lst) % 2:
                            groups.append([lst[-1]])
                    return groups

                # ---- DMA out: one DMA per row-region covering all 3 mb
                def y_out(img, c0, c1):
                    a, b2_ = W * c0, W * c1
                    dst = y_d[img * BLK:(img + 1) * BLK, :, a:b2_].rearrange(
                        "i p q -> p i q")
                    src_ = ym[img].rearrange("p (i q) -> p i q", q=SLEN)[
                        :, :, a:b2_]
                    nc.sync.dma_start(out=dst, in_=src_)

                for img in range(2):
                    for grp in pw_order(img):
                        pw_group(img, grp)
                    d = D_IMG[img]
                    dc = CHUNK * ((d + CHUNK - 1) // CHUNK)
                    y_out(img, dc, H)
                    y_out(img, 0, dc // 2)
                    y_out(img, dc // 2, dc)

    nc.compile()
    return nc


def _get_nc(reps=1):
    key = ("nc", reps)
    if key not in _cache:
        import concourse.bass as bass
        import concourse.bacc as bacc
        import concourse.tile as tile
        import concourse.mybir as mybir
        _cache[key] = _build((bass, bacc, tile, mybir), reps)
        if reps == 1:
            _cache["nc"] = _cache[key]
    return _cache[key]


def kernel(x: np.ndarray, w_dw: np.ndarray, w_pw: np.ndarray) -> np.ndarray:
    assert x.shape == (IMGS, C, H, W) and x.dtype == np.float32
    # ---- host-side quantization + folding (mirrors the fp32 reference)
    m = np.maximum(np.mean(np.abs(w_dw.reshape(C, -1)), axis=1,
                           dtype=np.float32), EPS)            # [C]
    u_dw = np.clip(np.round(w_dw[:, 0] * (1.0 / m)[:, None, None]), -1, 1)
    M_pw = max(np.mean(np.abs(w_pw), dtype=np.float32), np.float32(EPS))
    u_pw = np.clip(np.round(w_pw[:, :, 0, 0] * (1.0 / M_pw)), -1, 1)
    Wt = (u_pw * (m * np.float32(M_pw))[None, :]).astype(np.float16)  # [O,C]

    # lhsT layout: wt16[k_part, (kb mb m)] = Wt[mb*128+m, kb*128+k_part]
    wt4 = Wt.reshape(BLK, 128, BLK, 128)            # [mb, mo, kb, ki]
    wt16 = np.ascontiguousarray(
        wt4.transpose(3, 2, 0, 1).reshape(128, BLK * BLK * 128))

    # diag tiles: dg16[p, (b t m)] = u_dw[b*128+m, t] if p == m else 0
    u16 = u_dw.astype(np.float16).reshape(BLK, 128, 9)   # [b, c, t]
    dg4 = np.zeros((128, BLK, 9, 128), dtype=np.float16)
    idx = np.arange(128)
    dg4[idx, :, :, idx] = u16.transpose(1, 0, 2)
    dg16 = np.ascontiguousarray(dg4.reshape(128, BLK * 9 * 128))

    sc32 = np.ascontiguousarray(
        u_dw.astype(np.float32).reshape(BLK, 128, 9).transpose(1, 0, 2)
        .reshape(128, BLK * 9))

    xpad = np.zeros((IMGS, BLK, 128, WP, WP), dtype=np.float16)
    xpad[:, :, :, 1:H + 1, 1:W + 1] = \
        x.reshape(IMGS, BLK, 128, H, W).astype(np.float16)
    xpad = xpad.reshape(IMGS, BLK, 128, PADLEN)

    nc = _get_nc()
    in_maps = []
    for k in range(N_CORES):
        xin = np.ascontiguousarray(
            xpad[IMG_PER_CORE * k:IMG_PER_CORE * (k + 1)]
            .reshape(IMG_PER_CORE * BLK, 128, PADLEN))
        in_maps.append({"x": xin, "wt": wt16, "dg": dg16, "sc": sc32})
    _cache["last_in_maps"] = in_maps

    from concourse import bass_utils
    res = bass_utils.run_bass_kernel_spmd(
        nc, in_maps, list(range(N_CORES)), **_cache.get("run_kwargs", {}))
    _cache["last_results"] = res

    out = np.empty((IMGS, C, H, W), dtype=np.float32)
    for k in range(N_CORES):
        yk = res.results[k]["y"].reshape(IMG_PER_CORE, BLK, 128, H, W)
        out[IMG_PER_CORE * k:IMG_PER_CORE * (k + 1)] = \
            yk.astype(np.float32).reshape(IMG_PER_CORE, C, H, W)
    return out


# revision 6
# speedup vs baseline: 1.3699x; 1.3699x over previous
"""BiSeparableConv (ternary depthwise 3x3 + ternary pointwise 1x1) on 8 TRN2 cores.

Math (folded on host):
  m_dw[c]  = max(mean|w_dw[c]|, EPS)            per-channel depthwise scale
  u_dw     = clip(round(w_dw / m_dw), -1, 1)    ternary taps
  M_pw     = max(mean|w_pw|, EPS)               global pointwise scale
  u_pw     = clip(round(w_pw / M_pw), -1, 1)
  y[n,o,s] = sum_c Wt[o,c] * z[n,c,s]           Wt = M_pw * u_pw * m_dw[c]
  z[n,c,s] = sum_t u_dw[c,t] * x[n,c,s+d_t]     9-tap depthwise, pad=1

Device (per core, 2 images, fp16 compute, fp32 PSUM accum):
  - x host-padded to 58x58 rows (zero border), fp16.
  - depthwise split per image between DVE and PE:
      DVE : rows [0, D_IMG[img]) via 1 tensor_scalar mul + 8 fused
            scalar_tensor_tensor FMAs (one pass per tap)
      PE  : rows [D_IMG[img], 56) via diagonal-matrix matmuls
            (9 taps accumulated in PSUM), ACT copies PSUM->SBUF
  - pointwise: 3x3 blocked matmul (K=384) over chunks of 8 rows,
    PSUM chunk-pairs copied out by ACT; per-image chunks that depend
    only on PE-computed z are issued first (DVE gets maximum slack).
  - y written fp16, host upcasts to fp32.
"""

import numpy as np

# ---------------------------------------------------------------- constants
N_CORES = 8
IMGS = 16
IMG_PER_CORE = 2
C = 384
BLK = 3          # channel blocks of 128
H = W = 56
WP = 58          # padded row width / padded row count
PADLEN = WP * WP           # 3364
SLEN = H * W               # 3136
EPS = 1e-5

# DVE depthwise row count per (img, blk) unit; units 0-4 take rows
# [0, d), unit 5 takes the TAIL rows [56-d, 56) so the last-finishing
# DVE chain feeds the last-consumed pointwise chunk.
D_UNIT = (32, 32, 32, 24, 16, 8)
CHUNK = 8          # pointwise / PE-dw chunk rows

TAPS = [(dh, dw) for dh in range(3) for dw in range(3)]
DELTA = {t: WP * t[0] + t[1] for t in TAPS}

_cache = {}


def _build(nc_mod, reps=1):
    bass, bacc, tile, mybir = nc_mod
    f16 = mybir.dt.float16
    f32 = mybir.dt.float32
    ALU = mybir.AluOpType

    nc = bacc.Bacc(
        "TRN2", target_bir_lowering=False, debug=False, num_devices=N_CORES
    )

    x_d = nc.dram_tensor("x", [IMG_PER_CORE * BLK, 128, PADLEN], f16,
                         kind="ExternalInput")
    wt_d = nc.dram_tensor("wt", [128, BLK * BLK * 128], f16,
                          kind="ExternalInput")
    dg_d = nc.dram_tensor("dg", [128, BLK * 9 * 128], f16,
                          kind="ExternalInput")
    sc_d = nc.dram_tensor("sc", [128, BLK * 9], f32, kind="ExternalInput")
    y_d = nc.dram_tensor("y", [IMG_PER_CORE * BLK, 128, SLEN], f16,
                         kind="ExternalOutput")

    with tile.TileContext(nc) as tc:
        with (
            tc.tile_pool(name="xa", bufs=1) as xa_pool,
            tc.tile_pool(name="zz", bufs=1) as z_pool,
            tc.tile_pool(name="yy", bufs=1) as y_pool,
            tc.tile_pool(name="tmp", bufs=2) as tmp_pool,
            tc.tile_pool(name="wts", bufs=1) as w_pool,
            tc.tile_pool(name="dwps", bufs=2, space="PSUM") as dwps,
            tc.tile_pool(name="pwps", bufs=3, space="PSUM") as pwps,
        ):
            xa = [xa_pool.tile([128, PADLEN], f16, tag=f"xa{u}", name=f"xa{u}")
                  for u in range(6)]
            z = [z_pool.tile([128, PADLEN], f16, tag=f"z{u}", name=f"z{u}")
                 for u in range(6)]
            ym = [y_pool.tile([128, BLK * SLEN], f16, tag=f"ym{i}",
                              name=f"ym{i}") for i in range(2)]
            wt = w_pool.tile([128, BLK * BLK * 128], f16, tag="wt", name="wt")
            dg = w_pool.tile([128, BLK * 9 * 128], f16, tag="dg", name="dg")
            sc = w_pool.tile([128, BLK * 9], f32, tag="sc", name="sc")

            def scal(b, t):
                return sc[:, b * 9 + TAPS.index(t), None]

            def wt_ap(kb, mb):
                i = kb * BLK + mb
                return wt[:, 128 * i:128 * (i + 1)]

            def dg_ap(b, t):
                i = b * 9 + TAPS.index(t)
                return dg[:, 128 * i:128 * (i + 1)]

            def unit(img, b):
                return img * BLK + b

            # DVE row range per unit: head units take [0, d), the last
            # unit takes the tail [56-d, 56).
            def dve_rows(u):
                d = D_UNIT[u]
                return (H - d, H) if u == 5 else (0, d)

            # x DMA band-split row per unit (band1 = rows [0, xs))
            def xs_row(u):
                return 10 if u == 5 else D_UNIT[u] + 3

            for _rep in range(reps):
                # ---- DMA in: ordered for earliest PE/DVE starts
                DG = 9 * 128

                def dma_x(u, band):
                    xs = WP * xs_row(u)
                    if band == 1:
                        nc.sync.dma_start(out=xa[u][:, :xs], in_=x_d[u][:, :xs])
                    else:
                        nc.sync.dma_start(out=xa[u][:, xs:], in_=x_d[u][:, xs:])

                nc.sync.dma_start(out=dg[:, :DG], in_=dg_d[:, :DG])
                dma_x(0, 2)
                dma_x(0, 1)
                nc.sync.dma_start(out=dg[:, DG:2 * DG], in_=dg_d[:, DG:2 * DG])
                nc.sync.dma_start(out=sc[:], in_=sc_d[:])
                dma_x(1, 2)
                nc.sync.dma_start(out=dg[:, 2 * DG:], in_=dg_d[:, 2 * DG:])
                dma_x(1, 1)
                dma_x(2, 2)
                dma_x(2, 1)
                dma_x(3, 2)
                nc.sync.dma_start(out=wt[:], in_=wt_d[:])
                dma_x(4, 2)
                dma_x(5, 2)
                dma_x(5, 1)
                dma_x(3, 1)
                dma_x(4, 1)

                # PE warmup: burn the pstate ramp while x bands land
                wps = dwps.tile([128, 512], f32, tag="dwps", name="dwps")
                for wi in range(3):
                    nc.tensor.matmul(wps[:, :384], dg[:, :128],
                                     dg[:, :3 * 128],
                                     start=(wi == 0), stop=(wi == 2))

                # ---- DVE: ACT seeds tap0, then 8 (ts-mul + tt-add) pairs
                def dve_chain(u):
                    b = u % BLK
                    r0, r1 = dve_rows(u)
                    p0 = WP * r0
                    p1 = min(WP * r1, PADLEN - 2 * WP - 2)
                    zr = z[u][:, p0:p1]
                    t0 = TAPS[0]
                    dlt = DELTA[t0]
                    nc.scalar.mul(zr, xa[u][:, p0 + dlt:p1 + dlt],
                                  scal(b, t0))
                    for t in TAPS[1:]:
                        dlt = DELTA[t]
                        tmp = tmp_pool.tile([128, p1 - p0], f16, tag="tmp",
                                            name="tmp")
                        nc.vector.tensor_scalar_mul(
                            tmp[:], xa[u][:, p0 + dlt:p1 + dlt], scal(b, t))
                        nc.vector.tensor_tensor(zr, zr, tmp[:], ALU.add)

                for u in range(6):
                    dve_chain(u)

                # ---- PE: remaining dw rows via diagonal matmuls
                def pe_dw_chunk(u, lo, nrow):
                    b = u % BLK
                    x3 = xa[u].rearrange("p (h w) -> p h w", w=WP)
                    z3 = z[u].rearrange("p (h w) -> p h w", w=WP)
                    ps = dwps.tile([128, 512], f32, tag="dwps", name="dwps")
                    dst = ps[:, :nrow * W]
                    for i, t in enumerate(TAPS):
                        dh, dw = t
                        rhs = x3[:, lo + dh:lo + dh + nrow, dw:dw + W]
                        nc.tensor.matmul(dst, dg_ap(b, t), rhs,
                                         start=(i == 0), stop=(i == 8))
                    nc.scalar.copy(z3[:, lo:lo + nrow, 0:W], dst)

                for u in range(6):
                    d = D_UNIT[u]
                    if u == 5:
                        for lo in range(0, H - d, CHUNK):
                            pe_dw_chunk(u, lo, CHUNK)
                    else:
                        # band2-only full chunks first; band1-touching last
                        for lo in range(d + CHUNK, H, CHUNK):
                            pe_dw_chunk(u, lo, CHUNK)
                        pe_dw_chunk(u, d, CHUNK)

                # ---- pointwise: chunk pairs into 2-bank PSUM, one ACT copy
                def pw_group(img, chunks):
                    for mb in range(BLK):
                        ps = pwps.tile([128, 1024], f32, tag="pwps",
                                       name="pwps")
                        for half, r0 in enumerate(chunks):
                            nrow = min(CHUNK, H - r0)
                            dst = ps[:, 512 * half:512 * half + nrow * W]
                            for kb in range(BLK):
                                zk = z[unit(img, kb)].rearrange(
                                    "p (h w) -> p h w", w=WP)
                                rhs = zk[:, r0:r0 + nrow, 0:W]
                                nc.tensor.matmul(dst, wt_ap(kb, mb), rhs,
                                                 start=(kb == 0),
                                                 stop=(kb == 2))
                        r0 = chunks[0]
                        yo = mb * SLEN
                        if len(chunks) == 2:
                            src_ap = ps.rearrange("p (a q) -> p a q", q=512)[
                                :, 0:2, 0:CHUNK * W]
                            dst_ap = ym[img][
                                :, yo + W * r0:yo + W * r0 + 2 * CHUNK * W
                            ].rearrange("p (a q) -> p a q", q=CHUNK * W)
                            nc.scalar.copy(dst_ap, src_ap)
                        else:
                            dst1 = ym[img][:, yo + W * r0:yo + W * (r0 + CHUNK)]
                            nc.scalar.copy(dst1, ps[:, :CHUNK * W])

                # ---- DMA out: one DMA per row-region covering all 3 mb
                def y_out(img, c0, c1):
                    a, b2_ = W * c0, W * c1
                    dst = y_d[img * BLK:(img + 1) * BLK, :, a:b2_].rearrange(
                        "i p q -> p i q")
                    src_ = ym[img].rearrange("p (i q) -> p i q", q=SLEN)[
                        :, :, a:b2_]
                    nc.sync.dma_start(out=dst, in_=src_)

                # img0: PE-fed chunks first, DVE-fed last.
                for grp in ([40, 48], [32], [0, 8], [16, 24]):
                    pw_group(0, grp)
                y_out(0, 32, H)
                y_out(0, 0, 32)
                # img1: ordered by DVE-chain completion (u3, u4, u5-tail).
                for grp, reg in (([24, 32], None), ([40], (24, 48)),
                                 ([16], (16, 24)), ([0, 8], (0, 16)),
                                 ([48], (48, H))):
                    pw_group(1, grp)
                    if reg is not None:
                        y_out(1, reg[0], reg[1])

    nc.compile()
    return nc


def _get_nc(reps=1):
    key = ("nc", reps)
    if key not in _cache:
        import concourse.bass as bass
        import concourse.bacc as bacc
        import concourse.tile as tile
        import concourse.mybir as mybir
        _cache[key] = _build((bass, bacc, tile, mybir), reps)
        if reps == 1:
            _cache["nc"] = _cache[key]
    return _cache[key]


def kernel(x: np.ndarray, w_dw: np.ndarray, w_pw: np.ndarray) -> np.ndarray:
    assert x.shape == (IMGS, C, H, W) and x.dtype == np.float32
    # ---- host-side quantization + folding (mirrors the fp32 reference)
    m = np.maximum(np.mean(np.abs(w_dw.reshape(C, -1)), axis=1,
                           dtype=np.float32), EPS)            # [C]
    u_dw = np.clip(np.round(w_dw[:, 0] * (1.0 / m)[:, None, None]), -1, 1)
    M_pw = max(np.mean(np.abs(w_pw), dtype=np.float32), np.float32(EPS))
    u_pw = np.clip(np.round(w_pw[:, :, 0, 0] * (1.0 / M_pw)), -1, 1)
    Wt = (u_pw * (m * np.float32(M_pw))[None, :]).astype(np.float16)  # [O,C]

    # lhsT layout: wt16[k_part, (kb mb m)] = Wt[mb*128+m, kb*128+k_part]
    wt4 = Wt.reshape(BLK, 128, BLK, 128)            # [mb, mo, kb, ki]
    wt16 = np.ascontiguousarray(
        wt4.transpose(3, 2, 0, 1).reshape(128, BLK * BLK * 128))

    # diag tiles: dg16[p, (b t m)] = u_dw[b*128+m, t] if p == m else 0
    u16 = u_dw.astype(np.float16).reshape(BLK, 128, 9)   # [b, c, t]
    dg4 = np.zeros((128, BLK, 9, 128), dtype=np.float16)
    idx = np.arange(128)
    dg4[idx, :, :, idx] = u16.transpose(1, 0, 2)
    dg16 = np.ascontiguousarray(dg4.reshape(128, BLK * 9 * 128))

    sc32 = np.ascontiguousarray(
        u_dw.astype(np.float32).reshape(BLK, 128, 9).transpose(1, 0, 2)
        .reshape(128, BLK * 9))

    xpad = np.zeros((IMGS, BLK, 128, WP, WP), dtype=np.float16)
    xpad[:, :, :, 1:H + 1, 1:W + 1] = \
        x.reshape(IMGS, BLK, 128, H, W).astype(np.float16)
    xpad = xpad.reshape(IMGS, BLK, 128, PADLEN)

    nc = _get_nc()
    in_maps = []
    for k in range(N_CORES):
        xin = np.ascontiguousarray(
            xpad[IMG_PER_CORE * k:IMG_PER_CORE * (k + 1)]
            .reshape(IMG_PER_CORE * BLK, 128, PADLEN))
        in_maps.append({"x": xin, "wt": wt16, "dg": dg16, "sc": sc32})
    _cache["last_in_maps"] = in_maps

    from concourse import bass_utils
    res = bass_utils.run_bass_kernel_spmd(
        nc, in_maps, list(range(N_CORES)), **_cache.get("run_kwargs", {}))
    _cache["last_results"] = res

    out = np.empty((IMGS, C, H, W), dtype=np.float32)
    for k in range(N_CORES):
        yk = res.results[k]["y"].reshape(IMG_PER_CORE, BLK, 128, H, W)
        out[IMG_PER_CORE * k:IMG_PER_CORE * (k + 1)] = \
            yk.astype(np.float32).reshape(IMG_PER_CORE, C, H, W)
    return out
